# revision 42
# baseline (speedup 1.0000x reference)
# Multi-head attention kernel for Trainium2 (8 NeuronCores, SPMD).
#
# Problem (hardcoded): X[4, 2048, 1024], W_k/W_q/W_v/W_u[1024, 1024], b_u[1024]
#   K = (X @ W_k.T) * s ; Q = (X @ W_q.T) * s ; V = (X @ W_v.T) * s   (s = 1024**-0.25)
#   S = Q @ K.T per head (16 heads, head_dim 64); P = softmax(S); Y = P @ V
#   out = Y @ W_u.T + b_u
#
# Sharding: core c = (batch c//2, head-half c%2). Each core computes K/Q/V for
# its 8 heads over the full sequence of its batch and the matching 512-wide
# slice of the output projection, producing a PARTIAL output [2048, 1024]
# (feature-contraction split). Host unshard = add the two partials per batch
# (column-sharded projection reduce); b_u is added on the hh==0 core only.
#
# Design (from trace analysis): exp on the Scalar engine is the hard floor
# (~295us/core: 33.5M score elements at 1 elem/cycle/lane @1.2GHz, invariant
# under sharding), so the kernel keeps ACT saturated and fits the PE work
# underneath it. Weights arrive pre-scaled/pre-transposed/bf16 from the host.
# The attention q axis runs in 512-wide quarters with both heads of a pair
# packed into one [128, 1024] score PSUM tile so each EXP activation covers
# both heads (N=1024 amortizes the ~350-cycle ACTIVATE overhead). Softmax
# denominators ride as row 64 of the AV matmul (ones column in V); their
# reciprocal is computed partition-packed ([128, 8] via a DRAM bounce)
# instead of on a 1-partition row, which the DVE executes pathologically
# slowly. The output projection splits each contraction into two 64-row
# halves on alternating row groups (the hardware pulls row-disjoint weight
# loads ahead of in-flight matmuls, so the halves stream concurrently).
#
# Per-core layout (PE contracts on partitions):
#   X^T  [e, t]      bf16 from host
#   K^T/Q^T [128, t] per head pair: head A on partitions 0..63, head B 64..127
#   V    [t, h, 65]  token-major, 65th column = ones (softmax denominator)
#   Y^T  [128, 4, t] feature-major (pair -> partition block), normalized
#   out  [t, e]      partial projection, fp32

import numpy as np
import ml_dtypes

import concourse.bacc as bacc
import concourse.mybir as mybir
import concourse.tile as tile
from concourse.bass_utils import run_bass_kernel_spmd

FP32 = mybir.dt.float32
BF16 = mybir.dt.bfloat16
AF = mybir.ActivationFunctionType
BF16NP = ml_dtypes.bfloat16

P = 128
E = 1024            # embedding dim
T = 2048            # sequence length
HC = 8              # heads per core
S = 64              # head dim
ET = E // P         # 8 contraction tiles over e
KT = T // P         # 16 key tiles
NPAIR = HC // 2     # 4 head pairs per core
QW = 512            # query-quarter width
NQ = T // QW        # 4 query quarters
FC = NPAIR          # feature k-tiles for the output projection (4)
SCALE = float(E ** -0.25)

N_CORES = 8


def _chunks(total, step):
    return [(o, min(step, total - o)) for o in range(0, total, step)]


def build_nc():
    nc = bacc.Bacc("TRN2", target_bir_lowering=False, debug=False,
                   enable_asserts=False)

    xt = nc.dram_tensor("xt", [E, T], BF16, kind="ExternalInput").ap()
    wk = nc.dram_tensor("wk", [E, P * NPAIR], BF16, kind="ExternalInput").ap()
    wq = nc.dram_tensor("wq", [E, P * NPAIR], BF16, kind="ExternalInput").ap()
    wv = nc.dram_tensor("wv", [E, P * NPAIR], BF16, kind="ExternalInput").ap()
    wu = nc.dram_tensor("wu", [P * NPAIR, E], BF16, kind="ExternalInput").ap()
    bu = nc.dram_tensor("bu", [1, E], FP32, kind="ExternalInput").ap()
    out = nc.dram_tensor("out", [T, E], FP32, kind="ExternalOutput").ap()

    with tile.TileContext(nc) as tc:
        _build_kernel(tc, nc, xt, wk, wq, wv, wu, bu, out)
    nc.compile()
    return nc


def _build_kernel(tc, nc, xt, wk, wq, wv, wu, bu, out):
    with (
        tc.tile_pool(name="main", bufs=1) as mp,
        tc.tile_pool(name="psum", bufs=1, space="PSUM") as pspool,
        tc.tile_pool(name="dram", bufs=1, space="DRAM") as drampool,
    ):
        vv = mp.tile([P, KT, HC, S + 1], BF16, tag="vv", name="vv")
        yt = mp.tile([P, FC, T], BF16, tag="yt", name="yt")
        bub = mp.tile([P, E], FP32, tag="bub", name="bub")

        # --- per-pair K/Q weight load + projection (kt/qt [128, T]:
        # head 2j on partitions 0..63, head 2j+1 on 64..127)
        wts = {}

        def load_wkq(j):
            wkj = mp.tile([P, ET, P], BF16, tag="wkj", bufs=2, name=f"wk{j}")
            wqj = mp.tile([P, ET, P], BF16, tag="wqj", bufs=2, name=f"wq{j}")
            nc.sync.dma_start(
                wkj[:], wk[:, j * P:(j + 1) * P].rearrange(
                    "(ko p) m -> p ko m", p=P))
            nc.sync.dma_start(
                wqj[:], wq[:, j * P:(j + 1) * P].rearrange(
                    "(ko p) m -> p ko m", p=P))
            wts[j] = (wkj, wqj)

        def emit_proj_512(j, which, dst, t0):
            # one 512-token projection sub-block, contraction split into
            # 64-row halves on alternating row groups (the hardware pulls
            # row-disjoint weight loads ahead, so the halves stream
            # concurrently at ~2x a serial accumulation chain), packed into
            # one ps tile and combined by two DVE ops
            wb = wts[j][which]
            uname = "kq"[which]
            ps = pspool.tile([P, 1024], FP32, tag="ps", bufs=3,
                             name=f"ps{uname}{j}_{t0}")
            for k in range(ET):
                for h in range(2):
                    nc.tensor.matmul(
                        ps[:, h * 512:(h + 1) * 512],
                        lhsT=wb[h * S:(h + 1) * S, k, :],
                        rhs=xt_sb[h * S:(h + 1) * S, k, t0:t0 + 512],
                        start=(k == 0), stop=(k == ET - 1))
            tsum = mp.tile([P, 512], FP32, tag="osum", bufs=2,
                           name=f"pj{uname}{j}_{t0}")
            nc.vector.tensor_copy(out=tsum[:], in_=ps[:, 512:1024])
            nc.vector.tensor_add(out=dst[:, t0:t0 + 512],
                                 in0=ps[:, 0:512], in1=tsum[:])

        def emit_proj_one(j, which, dst):
            for t0 in range(0, T, 512):
                emit_proj_512(j, which, dst, t0)

        # --- X^T (bf16 direct from host), chunked per k-tile for early start
        xt_sb = mp.tile([P, ET, T], BF16, tag="xtb", name="xtb")
        load_wkq(0)
        for k in range(ET):
            nc.sync.dma_start(xt_sb[:, k, :], xt[k * P:(k + 1) * P, :])
        wvb = mp.tile([P, ET, P * NPAIR], BF16, tag="wvb", name="wvb")
        nc.sync.dma_start(wvb[:], wv.rearrange("(ko p) m -> p ko m", p=P))
        nc.sync.dma_start(bub[:], bu.to_broadcast([P, E]))

        # pair-0 K/Q projection emitted first, sub-block-interleaved so the
        # first scores only wait for the first kt/qt 512-token sub-blocks
        kq = {0: (mp.tile([P, T], BF16, tag="ktj", bufs=2, name="kt0"),
                  mp.tile([P, T], BF16, tag="qtj", bufs=2, name="qt0"))}
        for t0 in range(0, T, 512):
            emit_proj_512(0, 0, kq[0][0], t0)
            emit_proj_512(0, 1, kq[0][1], t0)

        # --- V projection -> vv[t, h, 0:64] token-major + ones column;
        # one token-tile per ps tile, contraction halves row-split like the
        # other projections
        for mt in range(KT):
            ps = pspool.tile([P, 1024], FP32, tag="ps", bufs=3,
                             name=f"psv{mt}")
            for k in range(ET):
                for h in range(2):
                    nc.tensor.matmul(
                        ps[:, h * 512:(h + 1) * 512],
                        lhsT=xt_sb[h * S:(h + 1) * S, k,
                                   mt * P:(mt + 1) * P],
                        rhs=wvb[h * S:(h + 1) * S, k, :],
                        start=(k == 0), stop=(k == ET - 1))
            vs = mp.tile([P, 512], FP32, tag="osum", bufs=2,
                         name=f"vs{mt}")
            nc.vector.tensor_copy(out=vs[:], in_=ps[:, 512:1024])
            nc.vector.tensor_add(
                out=vv[:, mt, :, 0:S],
                in0=ps[:, 0:512].rearrange("p (h s) -> p h s", s=S),
                in1=vs[:].rearrange("p (h s) -> p h s", s=S))
            nc.vector.memset(vv[:, mt, :, S:S + 1], 1.0)

        # output projection weights, DMA'd early so the tail never waits
        wub = mp.tile([P, FC, E], BF16, tag="wub", name="wub")
        nc.sync.dma_start(wub[:], wu.rearrange("(ko p) m -> p ko m", p=P))

        def emit_out_tile(m):
            # output projection token tile; contraction split into 64-row
            # halves on alternating row groups (halves stream concurrently,
            # ~2x a serial accumulation chain), packed into one ps tile
            ot = mp.tile([P, E], FP32, tag="ot", bufs=2, name=f"ot{m}")
            for n0 in (0, 512):
                ph = pspool.tile([P, 1024], FP32, tag="ps", bufs=3,
                                 name=f"po{m}_{n0}")
                for k in range(FC):
                    for h in range(2):
                        nc.tensor.matmul(
                            ph[:, h * 512:(h + 1) * 512],
                            lhsT=yt[h * S:(h + 1) * S, k,
                                    m * P:(m + 1) * P],
                            rhs=wub[h * S:(h + 1) * S, k, n0:n0 + 512],
                            start=(k == 0), stop=(k == FC - 1))
                tsum = mp.tile([P, 512], FP32, tag="osum", bufs=2,
                               name=f"os{m}_{n0}")
                nc.vector.tensor_add(out=tsum[:], in0=ph[:, 0:512],
                                     in1=bub[:, n0:n0 + 512])
                nc.vector.tensor_add(out=ot[:, n0:n0 + 512],
                                     in0=ph[:, 512:1024], in1=tsum[:])
            nc.sync.dma_start(out[m * P:(m + 1) * P, :], ot[:])

        # --- head pairs; per pair, 4 query-quarters of 512
        for j in range(NPAIR):
            ktj, qtj = kq.pop(j)
            for qq in range(NQ):
                q0 = qq * QW
                # AV accumulators: one PSUM bank per parity, held over the
                # full key loop (ones column gives the denominator as row 64)
                avs = [pspool.tile([P, QW], FP32, tag="av", bufs=2,
                                   name=f"av{j}_{qq}_{par}")
                       for par in range(2)]
                for i in range(KT):
                    ps = pspool.tile([P, 1024], FP32, tag="ps", bufs=3,
                                     name=f"s{j}_{qq}_{i}")
                    for par in range(2):
                        lo = par * S
                        nc.tensor.matmul(
                            ps[:, par * QW:(par + 1) * QW],
                            lhsT=ktj[lo:lo + S, i * P:(i + 1) * P],
                            rhs=qtj[lo:lo + S, q0:q0 + QW],
                            start=True, stop=True)
                    pt = mp.tile([P, 1024], BF16, tag="pt", bufs=16,
                                 name=f"p{j}_{qq}_{i}")
                    nc.scalar.activation(pt[:], ps[:], AF.Exp)
                    for par in range(2):
                        nc.tensor.matmul(
                            avs[par][0:S + 1, :],
                            lhsT=vv[:, i, 2 * j + par, :],
                            rhs=pt[:, par * QW:(par + 1) * QW],
                            start=(i == 0), stop=(i == KT - 1))
                # prefetch next pair's K/Q projection into the PE's slack,
                # two 512-token sub-blocks per quarter boundary so each
                # block (~2x 1.7us) fits under the pending-exp backlog
                if j + 1 < NPAIR:
                    if qq == 0:
                        load_wkq(j + 1)
                        kq[j + 1] = (
                            mp.tile([P, T], BF16, tag="ktj", bufs=2,
                                    name=f"kt{j+1}"),
                            mp.tile([P, T], BF16, tag="qtj", bufs=2,
                                    name=f"qt{j+1}"))
                        emit_proj_512(j + 1, 0, kq[j + 1][0], 0)
                        emit_proj_512(j + 1, 0, kq[j + 1][0], 512)
                    elif qq == 1:
                        emit_proj_512(j + 1, 0, kq[j + 1][0], 1024)
                        emit_proj_512(j + 1, 0, kq[j + 1][0], 1536)
                    elif qq == 2:
                        emit_proj_512(j + 1, 1, kq[j + 1][1], 0)
                        emit_proj_512(j + 1, 1, kq[j + 1][1], 512)
                    elif qq == 3:
                        emit_proj_512(j + 1, 1, kq[j + 1][1], 1024)
                        emit_proj_512(j + 1, 1, kq[j + 1][1], 1536)

                # normalize: evict AV banks, batch both parities' denominator
                # rows into a [128, 8] partition-packed reciprocal via a DRAM
                # bounce, broadcast back, multiply into yt
                yraws = []
                for par in range(2):
                    yraw = mp.tile([S + 1, QW], FP32, tag=f"yraw{par}",
                                   bufs=2, name=f"yraw{j}_{qq}_{par}")
                    nc.vector.tensor_copy(out=yraw[:], in_=avs[par][0:S + 1, :])
                    yraws.append(yraw)
                db = drampool.tile([1, 1024], FP32, tag="db", bufs=4,
                                   name=f"db{j}_{qq}")
                for par in range(2):
                    nc.sync.dma_start(db[:, par * QW:(par + 1) * QW],
                                      yraws[par][S:S + 1, :])
                rin = mp.tile([P, 8], FP32, tag="rin", bufs=2,
                              name=f"rin{j}_{qq}")
                nc.sync.dma_start(
                    rin[:], db[0:1, :].rearrange("a (p f) -> (a p) f", p=P))
                rcp = mp.tile([P, 8], FP32, tag="rcp", bufs=2,
                              name=f"rcp{j}_{qq}")
                nc.vector.reciprocal_approx_fast(out=rcp[:], in_=rin[:])
                db2 = drampool.tile([1, 1024], FP32, tag="db2", bufs=4,
                                    name=f"db2{j}_{qq}")
                nc.sync.dma_start(
                    db2[0:1, :].rearrange("a (p f) -> (a p) f", p=P), rcp[:])
                for par in range(2):
                    rbc = mp.tile([S, QW], FP32, tag="rbc", bufs=2,
                                  name=f"rbc{j}_{qq}_{par}")
                    nc.sync.dma_start(
                        rbc[:],
                        db2[0:1, par * QW:(par + 1) * QW].to_broadcast(
                            [S, QW]))
                    if par == 0:
                        nc.vector.tensor_mul(out=yt[0:S, j, q0:q0 + QW],
                                             in0=yraws[par][0:S, :],
                                             in1=rbc[:])
                    else:
                        tmp = mp.tile([S, QW], BF16, tag="tmp", bufs=2,
                                      name=f"tmp{j}_{qq}")
                        nc.vector.tensor_mul(out=tmp[:],
                                             in0=yraws[par][0:S, :],
                                             in1=rbc[:])
                        nc.sync.dma_start(yt[S:P, j, q0:q0 + QW], tmp[:])

        # --- output projection out[q, e'] = Y^T.T @ W_u^T + b_u (partial)
        for m in range(T // P):
            emit_out_tile(m)


_NC_CACHE = {}


def _get_nc():
    if "nc" not in _NC_CACHE:
        _NC_CACHE["nc"] = build_nc()
    return _NC_CACHE["nc"]


def make_in_maps(X, W_k, W_q, W_v, W_u, b_u):
    X = np.asarray(X, np.float32)
    b = X.shape[0]
    HW = P * NPAIR  # 512 features per head-half
    # pre-transpose, pre-scale, cast to bf16 on host (same numerics as the
    # on-device scale+cast it replaces)
    wk_t = (np.asarray(W_k, np.float32).T * SCALE).astype(BF16NP)
    wq_t = (np.asarray(W_q, np.float32).T * SCALE).astype(BF16NP)
    wv_t = (np.asarray(W_v, np.float32).T * SCALE).astype(BF16NP)
    wu_t = np.asarray(W_u, np.float32).T.astype(BF16NP)
    bu2 = np.ascontiguousarray(np.asarray(b_u, np.float32).reshape(1, E))
    bu_zero = np.zeros((1, E), np.float32)
    wk_s = [np.ascontiguousarray(wk_t[:, hh * HW:(hh + 1) * HW])
            for hh in range(2)]
    wq_s = [np.ascontiguousarray(wq_t[:, hh * HW:(hh + 1) * HW])
            for hh in range(2)]
    wv_s = [np.ascontiguousarray(wv_t[:, hh * HW:(hh + 1) * HW])
            for hh in range(2)]
    wu_s = [np.ascontiguousarray(wu_t[hh * HW:(hh + 1) * HW, :])
            for hh in range(2)]
    xts = [np.ascontiguousarray(X[bi].T).astype(BF16NP) for bi in range(b)]
    in_maps = []
    for c in range(N_CORES):
        bi, hh = c // 2, c % 2
        in_maps.append({
            "xt": xts[bi],
            "wk": wk_s[hh], "wq": wq_s[hh], "wv": wv_s[hh],
            "wu": wu_s[hh],
            "bu": bu2 if hh == 0 else bu_zero,
        })
    return in_maps


def run(inputs, trace=False, **kwargs):
    """Run on hardware; returns (full output, BassKernelResults)."""
    X = np.asarray(inputs["X"], np.float32)
    b, t, e = X.shape
    nc = _get_nc()
    in_maps = make_in_maps(X, inputs["W_k"], inputs["W_q"], inputs["W_v"],
                           inputs["W_u"], inputs["b_u"])
    res = run_bass_kernel_spmd(nc, in_maps, core_ids=list(range(N_CORES)),
                               trace=trace, **kwargs)
    full = np.empty((b, t, e), np.float32)
    for bi in range(b):
        full[bi] = res.results[2 * bi]["out"] + res.results[2 * bi + 1]["out"]
    return full, res


def kernel(**inputs):
    full, _ = run(inputs)
    return full


# revision 43
# speedup vs baseline: 1.0992x; 1.0992x over previous
# Multi-head attention kernel for Trainium2 (8 NeuronCores, SPMD).
#
# Problem (hardcoded): X[4, 2048, 1024], W_k/W_q/W_v/W_u[1024, 1024], b_u[1024]
#   K = (X @ W_k.T) * s ; Q = (X @ W_q.T) * s ; V = (X @ W_v.T) * s   (s = 1024**-0.25)
#   S = Q @ K.T per head (16 heads, head_dim 64); P = softmax(S); Y = P @ V
#   out = Y @ W_u.T + b_u
#
# Sharding: core c = (batch c//2, head-half c%2). Each core computes K/Q/V for
# its 8 heads over the full sequence of its batch and the matching 512-wide
# slice of the output projection, producing a PARTIAL output [2048, 1024]
# (feature-contraction split). Host unshard = add the two partials per batch
# (column-sharded projection reduce); b_u is added on the hh==0 core only.
#
# Design (from trace analysis): exp on the Scalar engine is the hard floor
# (~295us/core: 33.5M score elements at 1 elem/cycle/lane @1.2GHz, invariant
# under sharding), so the kernel keeps ACT saturated and fits the PE work
# underneath it. Weights arrive pre-scaled/pre-transposed/bf16 from the host.
# The attention q axis runs in 512-wide quarters with both heads of a pair
# packed into one [128, 1024] score PSUM tile so each EXP activation covers
# both heads (N=1024 amortizes the ~350-cycle ACTIVATE overhead). Softmax
# denominators ride as row 64 of the AV matmul (ones column in V); their
# reciprocal is computed partition-packed ([128, 8] via a DRAM bounce)
# instead of on a 1-partition row, which the DVE executes pathologically
# slowly. The output projection splits each contraction into two 64-row
# halves on alternating row groups (the hardware pulls row-disjoint weight
# loads ahead of in-flight matmuls, so the halves stream concurrently).
#
# Per-core layout (PE contracts on partitions):
#   X^T  [e, t]      bf16 from host
#   K^T/Q^T [128, t] per head pair: head A on partitions 0..63, head B 64..127
#   V    [t, h, 65]  token-major, 65th column = ones (softmax denominator)
#   Y^T  [128, 4, t] feature-major (pair -> partition block), normalized
#   out  [t, e]      partial projection, fp32

import numpy as np
import ml_dtypes

import concourse.bacc as bacc
import concourse.mybir as mybir
import concourse.tile as tile
from concourse.bass_utils import run_bass_kernel_spmd

FP32 = mybir.dt.float32
BF16 = mybir.dt.bfloat16
AF = mybir.ActivationFunctionType
BF16NP = ml_dtypes.bfloat16

P = 128
E = 1024            # embedding dim
T = 2048            # sequence length
HC = 8              # heads per core
S = 64              # head dim
ET = E // P         # 8 contraction tiles over e
KT = T // P         # 16 key tiles
NPAIR = HC // 2     # 4 head pairs per core
QW = 512            # query-quarter width
NQ = T // QW        # 4 query quarters
FC = NPAIR          # feature k-tiles for the output projection (4)
SCALE = float(E ** -0.25)

N_CORES = 8


def _chunks(total, step):
    return [(o, min(step, total - o)) for o in range(0, total, step)]


def build_nc():
    nc = bacc.Bacc("TRN2", target_bir_lowering=False, debug=False,
                   enable_asserts=False)

    xt = nc.dram_tensor("xt", [E, T], BF16, kind="ExternalInput").ap()
    wk = nc.dram_tensor("wk", [E, P * NPAIR], BF16, kind="ExternalInput").ap()
    wq = nc.dram_tensor("wq", [E, P * NPAIR], BF16, kind="ExternalInput").ap()
    wv = nc.dram_tensor("wv", [E, P * NPAIR], BF16, kind="ExternalInput").ap()
    wu = nc.dram_tensor("wu", [P * NPAIR, E], BF16, kind="ExternalInput").ap()
    bu = nc.dram_tensor("bu", [1, E], FP32, kind="ExternalInput").ap()
    out = nc.dram_tensor("out", [T, E], FP32, kind="ExternalOutput").ap()

    with tile.TileContext(nc) as tc:
        _build_kernel(tc, nc, xt, wk, wq, wv, wu, bu, out)
    nc.compile()
    return nc


def _build_kernel(tc, nc, xt, wk, wq, wv, wu, bu, out):
    with (
        tc.tile_pool(name="main", bufs=1) as mp,
        tc.tile_pool(name="psum", bufs=1, space="PSUM") as pspool,
        tc.tile_pool(name="dram", bufs=1, space="DRAM") as drampool,
    ):
        vv = mp.tile([P, KT, HC, S + 1], BF16, tag="vv", name="vv")
        yt = mp.tile([P, FC, T], BF16, tag="yt", name="yt")
        bub = mp.tile([P, E], FP32, tag="bub", name="bub")

        # --- per-pair K/Q weight load + projection (kt/qt [128, T]:
        # head 2j on partitions 0..63, head 2j+1 on 64..127)
        wts = {}

        def load_wkq(j):
            wkj = mp.tile([P, ET, P], BF16, tag="wkj", bufs=2, name=f"wk{j}")
            wqj = mp.tile([P, ET, P], BF16, tag="wqj", bufs=2, name=f"wq{j}")
            nc.sync.dma_start(
                wkj[:], wk[:, j * P:(j + 1) * P].rearrange(
                    "(ko p) m -> p ko m", p=P))
            nc.sync.dma_start(
                wqj[:], wq[:, j * P:(j + 1) * P].rearrange(
                    "(ko p) m -> p ko m", p=P))
            wts[j] = (wkj, wqj)

        def emit_proj_chunk(j, which, dst, t0):
            wb = wts[j][which]
            uname = "kq"[which]
            ps = pspool.tile([P, 1024], FP32, tag="ps", bufs=3,
                             name=f"ps{uname}{j}_{t0}")
            for n0 in (0, 512):
                for k in range(ET):
                    nc.tensor.matmul(
                        ps[:, n0:n0 + 512],
                        lhsT=wb[:, k, :],
                        rhs=xt_sb[:, k, t0 + n0:t0 + n0 + 512],
                        start=(k == 0), stop=(k == ET - 1))
            nc.vector.tensor_copy(out=dst[:, t0:t0 + 1024], in_=ps[:])

        def emit_proj_one(j, which, dst):
            for t0 in (0, 1024):
                emit_proj_chunk(j, which, dst, t0)

        # --- X^T (bf16 direct from host), chunked per k-tile for early start
        xt_sb = mp.tile([P, ET, T], BF16, tag="xtb", name="xtb")
        load_wkq(0)
        for k in range(ET):
            nc.sync.dma_start(xt_sb[:, k, :], xt[k * P:(k + 1) * P, :])
        wvb = mp.tile([P, ET, P * NPAIR], BF16, tag="wvb", name="wvb")
        nc.sync.dma_start(wvb[:], wv.rearrange("(ko p) m -> p ko m", p=P))
        nc.sync.dma_start(bub[:], bu.to_broadcast([P, E]))

        # pair-0 K/Q projection emitted first, chunk-interleaved so the
        # first scores only wait for (kt c0, qt c0)
        kq = {0: (mp.tile([P, T], BF16, tag="ktj", bufs=2, name="kt0"),
                  mp.tile([P, T], BF16, tag="qtj", bufs=2, name="qt0"))}
        emit_proj_chunk(0, 0, kq[0][0], 0)
        emit_proj_chunk(0, 1, kq[0][1], 0)
        emit_proj_chunk(0, 0, kq[0][0], 1024)
        emit_proj_chunk(0, 1, kq[0][1], 1024)

        # --- V projection -> vv[t, h, 0:64] token-major + ones column,
        # two token-tiles per PSUM tile
        for mt in range(0, KT, 2):
            ps = pspool.tile([P, 1024], FP32, tag="ps", bufs=3,
                             name=f"psv{mt}")
            for sub in range(2):
                for k in range(ET):
                    nc.tensor.matmul(
                        ps[:, sub * 512:(sub + 1) * 512],
                        lhsT=xt_sb[:, k, (mt + sub) * P:(mt + sub + 1) * P],
                        rhs=wvb[:, k, :],
                        start=(k == 0), stop=(k == ET - 1))
            nc.vector.tensor_copy(
                out=vv[:, mt:mt + 2, :, 0:S],
                in_=ps[:].rearrange("p (m h s) -> p m h s", m=2, s=S))
            nc.vector.memset(vv[:, mt:mt + 2, :, S:S + 1], 1.0)

        # output projection weights, DMA'd early so the tail never waits
        wub = mp.tile([P, FC, E], BF16, tag="wub", name="wub")
        nc.sync.dma_start(wub[:], wu.rearrange("(ko p) m -> p ko m", p=P))

        def emit_out_tile(m):
            # output projection token tile; contraction split into 64-row
            # halves on alternating row groups (halves stream concurrently,
            # ~2x a serial accumulation chain), packed into one ps tile
            ot = mp.tile([P, E], FP32, tag="ot", bufs=2, name=f"ot{m}")
            for n0 in (0, 512):
                ph = pspool.tile([P, 1024], FP32, tag="ps", bufs=3,
                                 name=f"po{m}_{n0}")
                for k in range(FC):
                    for h in range(2):
                        nc.tensor.matmul(
                            ph[:, h * 512:(h + 1) * 512],
                            lhsT=yt[h * S:(h + 1) * S, k,
                                    m * P:(m + 1) * P],
                            rhs=wub[h * S:(h + 1) * S, k, n0:n0 + 512],
                            start=(k == 0), stop=(k == FC - 1))
                tsum = mp.tile([P, 512], FP32, tag="osum", bufs=2,
                               name=f"os{m}_{n0}")
                nc.vector.tensor_add(out=tsum[:], in0=ph[:, 0:512],
                                     in1=bub[:, n0:n0 + 512])
                nc.vector.tensor_add(out=ot[:, n0:n0 + 512],
                                     in0=ph[:, 512:1024], in1=tsum[:])
            nc.sync.dma_start(out[m * P:(m + 1) * P, :], ot[:])

        # --- head pairs; per pair, 4 query-quarters of 512
        for j in range(NPAIR):
            ktj, qtj = kq.pop(j)
            for qq in range(NQ):
                q0 = qq * QW
                # AV accumulators: one PSUM bank per parity, held over the
                # full key loop (ones column gives the denominator as row 64)
                avs = [pspool.tile([P, QW], FP32, tag="av", bufs=2,
                                   name=f"av{j}_{qq}_{par}")
                       for par in range(2)]
                for i in range(KT):
                    ps = pspool.tile([P, 1024], FP32, tag="ps", bufs=3,
                                     name=f"s{j}_{qq}_{i}")
                    for par in range(2):
                        lo = par * S
                        nc.tensor.matmul(
                            ps[:, par * QW:(par + 1) * QW],
                            lhsT=ktj[lo:lo + S, i * P:(i + 1) * P],
                            rhs=qtj[lo:lo + S, q0:q0 + QW],
                            start=True, stop=True)
                    pt = mp.tile([P, 1024], BF16, tag="pt", bufs=16,
                                 name=f"p{j}_{qq}_{i}")
                    nc.scalar.activation(pt[:], ps[:], AF.Exp)
                    for par in range(2):
                        nc.tensor.matmul(
                            avs[par][0:S + 1, :],
                            lhsT=vv[:, i, 2 * j + par, :],
                            rhs=pt[:, par * QW:(par + 1) * QW],
                            start=(i == 0), stop=(i == KT - 1))
                # prefetch next pair's K/Q projection into the PE's slack,
                # split across two quarter boundaries (kt after qq0, qt
                # after qq1) so each block is half the size
                if j + 1 < NPAIR:
                    if qq == 0:
                        load_wkq(j + 1)
                        kq[j + 1] = (
                            mp.tile([P, T], BF16, tag="ktj", bufs=2,
                                    name=f"kt{j+1}"),
                            mp.tile([P, T], BF16, tag="qtj", bufs=2,
                                    name=f"qt{j+1}"))
                        emit_proj_one(j + 1, 0, kq[j + 1][0])
                    elif qq == 1:
                        emit_proj_one(j + 1, 1, kq[j + 1][1])

                # normalize: evict AV banks, batch both parities' denominator
                # rows into a [128, 8] partition-packed reciprocal via a DRAM
                # bounce, broadcast back, multiply into yt
                yraws = []
                for par in range(2):
                    yraw = mp.tile([S + 1, QW], FP32, tag=f"yraw{par}",
                                   bufs=2, name=f"yraw{j}_{qq}_{par}")
                    nc.vector.tensor_copy(out=yraw[:], in_=avs[par][0:S + 1, :])
                    yraws.append(yraw)
                db = drampool.tile([1, 1024], FP32, tag="db", bufs=4,
                                   name=f"db{j}_{qq}")
                for par in range(2):
                    nc.sync.dma_start(db[:, par * QW:(par + 1) * QW],
                                      yraws[par][S:S + 1, :])
                rin = mp.tile([P, 8], FP32, tag="rin", bufs=2,
                              name=f"rin{j}_{qq}")
                nc.sync.dma_start(
                    rin[:], db[0:1, :].rearrange("a (p f) -> (a p) f", p=P))
                rcp = mp.tile([P, 8], FP32, tag="rcp", bufs=2,
                              name=f"rcp{j}_{qq}")
                nc.vector.reciprocal_approx_fast(out=rcp[:], in_=rin[:])
                db2 = drampool.tile([1, 1024], FP32, tag="db2", bufs=4,
                                    name=f"db2{j}_{qq}")
                nc.sync.dma_start(
                    db2[0:1, :].rearrange("a (p f) -> (a p) f", p=P), rcp[:])
                for par in range(2):
                    rbc = mp.tile([S, QW], FP32, tag="rbc", bufs=2,
                                  name=f"rbc{j}_{qq}_{par}")
                    nc.sync.dma_start(
                        rbc[:],
                        db2[0:1, par * QW:(par + 1) * QW].to_broadcast(
                            [S, QW]))
                    if par == 0:
                        nc.vector.tensor_mul(out=yt[0:S, j, q0:q0 + QW],
                                             in0=yraws[par][0:S, :],
                                             in1=rbc[:])
                    else:
                        tmp = mp.tile([S, QW], BF16, tag="tmp", bufs=2,
                                      name=f"tmp{j}_{qq}")
                        nc.vector.tensor_mul(out=tmp[:],
                                             in0=yraws[par][0:S, :],
                                             in1=rbc[:])
                        nc.sync.dma_start(yt[S:P, j, q0:q0 + QW], tmp[:])

        # --- output projection out[q, e'] = Y^T.T @ W_u^T + b_u (partial)
        for m in range(T // P):
            emit_out_tile(m)


_NC_CACHE = {}


def _get_nc():
    if "nc" not in _NC_CACHE:
        _NC_CACHE["nc"] = build_nc()
    return _NC_CACHE["nc"]


def make_in_maps(X, W_k, W_q, W_v, W_u, b_u):
    X = np.asarray(X, np.float32)
    b = X.shape[0]
    HW = P * NPAIR  # 512 features per head-half
    # pre-transpose, pre-scale, cast to bf16 on host (same numerics as the
    # on-device scale+cast it replaces)
    wk_t = (np.asarray(W_k, np.float32).T * SCALE).astype(BF16NP)
    wq_t = (np.asarray(W_q, np.float32).T * SCALE).astype(BF16NP)
    wv_t = (np.asarray(W_v, np.float32).T * SCALE).astype(BF16NP)
    wu_t = np.asarray(W_u, np.float32).T.astype(BF16NP)
    bu2 = np.ascontiguousarray(np.asarray(b_u, np.float32).reshape(1, E))
    bu_zero = np.zeros((1, E), np.float32)
    wk_s = [np.ascontiguousarray(wk_t[:, hh * HW:(hh + 1) * HW])
            for hh in range(2)]
    wq_s = [np.ascontiguousarray(wq_t[:, hh * HW:(hh + 1) * HW])
            for hh in range(2)]
    wv_s = [np.ascontiguousarray(wv_t[:, hh * HW:(hh + 1) * HW])
            for hh in range(2)]
    wu_s = [np.ascontiguousarray(wu_t[hh * HW:(hh + 1) * HW, :])
            for hh in range(2)]
    xts = [np.ascontiguousarray(X[bi].T).astype(BF16NP) for bi in range(b)]
    in_maps = []
    for c in range(N_CORES):
        bi, hh = c // 2, c % 2
        in_maps.append({
            "xt": xts[bi],
            "wk": wk_s[hh], "wq": wq_s[hh], "wv": wv_s[hh],
            "wu": wu_s[hh],
            "bu": bu2 if hh == 0 else bu_zero,
        })
    return in_maps


def run(inputs, trace=False, **kwargs):
    """Run on hardware; returns (full output, BassKernelResults)."""
    X = np.asarray(inputs["X"], np.float32)
    b, t, e = X.shape
    nc = _get_nc()
    in_maps = make_in_maps(X, inputs["W_k"], inputs["W_q"], inputs["W_v"],
                           inputs["W_u"], inputs["b_u"])
    res = run_bass_kernel_spmd(nc, in_maps, core_ids=list(range(N_CORES)),
                               trace=trace, **kwargs)
    full = np.empty((b, t, e), np.float32)
    for bi in range(b):
        full[bi] = res.results[2 * bi]["out"] + res.results[2 * bi + 1]["out"]
    return full, res


def kernel(**inputs):
    full, _ = run(inputs)
    return full


# revision 45
# speedup vs baseline: 1.1000x; 1.0007x over previous
# Multi-head attention kernel for Trainium2 (8 NeuronCores, SPMD).
#
# Problem (hardcoded): X[4, 2048, 1024], W_k/W_q/W_v/W_u[1024, 1024], b_u[1024]
#   K = (X @ W_k.T) * s ; Q = (X @ W_q.T) * s ; V = (X @ W_v.T) * s   (s = 1024**-0.25)
#   S = Q @ K.T per head (16 heads, head_dim 64); P = softmax(S); Y = P @ V
#   out = Y @ W_u.T + b_u
#
# Sharding: core c = (batch c//2, head-half c%2). Each core computes K/Q/V for
# its 8 heads over the full sequence of its batch and the matching 512-wide
# slice of the output projection, producing a PARTIAL output [2048, 1024]
# (feature-contraction split). Host unshard = add the two partials per batch
# (column-sharded projection reduce); b_u is added on the hh==0 core only.
#
# Design (from trace analysis): exp on the Scalar engine is the hard floor
# (~295us/core: 33.5M score elements at 1 elem/cycle/lane @1.2GHz, invariant
# under sharding), so the kernel keeps ACT saturated and fits the PE work
# underneath it. Weights arrive pre-scaled/pre-transposed/bf16 from the host.
# The attention q axis runs in 512-wide quarters with both heads of a pair
# packed into one [128, 1024] score PSUM tile so each EXP activation covers
# both heads (N=1024 amortizes the ~350-cycle ACTIVATE overhead). Softmax
# denominators ride as row 64 of the AV matmul (ones column in V); their
# reciprocal is computed partition-packed ([128, 8] via a DRAM bounce)
# instead of on a 1-partition row, which the DVE executes pathologically
# slowly. The output projection splits each contraction into two 64-row
# halves on alternating row groups (the hardware pulls row-disjoint weight
# loads ahead of in-flight matmuls, so the halves stream concurrently).
#
# Per-core layout (PE contracts on partitions):
#   X^T  [e, t]      bf16 from host
#   K^T/Q^T [128, t] per head pair: head A on partitions 0..63, head B 64..127
#   V    [t, h, 65]  token-major, 65th column = ones (softmax denominator)
#   Y^T  [128, 4, t] feature-major (pair -> partition block), normalized
#   out  [t, e]      partial projection, fp32

import numpy as np
import ml_dtypes

import concourse.bacc as bacc
import concourse.mybir as mybir
import concourse.tile as tile
from concourse.bass_utils import run_bass_kernel_spmd

FP32 = mybir.dt.float32
BF16 = mybir.dt.bfloat16
AF = mybir.ActivationFunctionType
BF16NP = ml_dtypes.bfloat16

P = 128
E = 1024            # embedding dim
T = 2048            # sequence length
HC = 8              # heads per core
S = 64              # head dim
ET = E // P         # 8 contraction tiles over e
KT = T // P         # 16 key tiles
NPAIR = HC // 2     # 4 head pairs per core
QW = 512            # query-quarter width
NQ = T // QW        # 4 query quarters
FC = NPAIR          # feature k-tiles for the output projection (4)
SCALE = float(E ** -0.25)

N_CORES = 8


def _chunks(total, step):
    return [(o, min(step, total - o)) for o in range(0, total, step)]


def build_nc():
    nc = bacc.Bacc("TRN2", target_bir_lowering=False, debug=False,
                   enable_asserts=False)

    xt = nc.dram_tensor("xt", [E, T], BF16, kind="ExternalInput").ap()
    wk = nc.dram_tensor("wk", [E, P * NPAIR], BF16, kind="ExternalInput").ap()
    wq = nc.dram_tensor("wq", [E, P * NPAIR], BF16, kind="ExternalInput").ap()
    wv = nc.dram_tensor("wv", [E, P * NPAIR], BF16, kind="ExternalInput").ap()
    wu = nc.dram_tensor("wu", [P * NPAIR, E], BF16, kind="ExternalInput").ap()
    bu = nc.dram_tensor("bu", [1, E], FP32, kind="ExternalInput").ap()
    out = nc.dram_tensor("out", [T, E], FP32, kind="ExternalOutput").ap()

    with tile.TileContext(nc) as tc:
        _build_kernel(tc, nc, xt, wk, wq, wv, wu, bu, out)
    nc.compile()
    return nc


def _build_kernel(tc, nc, xt, wk, wq, wv, wu, bu, out):
    with (
        tc.tile_pool(name="main", bufs=1) as mp,
        tc.tile_pool(name="psum", bufs=1, space="PSUM") as pspool,
        tc.tile_pool(name="dram", bufs=1, space="DRAM") as drampool,
    ):
        vv = mp.tile([P, KT, HC, S + 1], BF16, tag="vv", name="vv")
        yt = mp.tile([P, FC, T], BF16, tag="yt", name="yt")
        bub = mp.tile([P, E], FP32, tag="bub", name="bub")

        # --- per-pair K/Q weight load + projection (kt/qt [128, T]:
        # head 2j on partitions 0..63, head 2j+1 on 64..127)
        wts = {}

        def load_wkq(j):
            wkj = mp.tile([P, ET, P], BF16, tag="wkj", bufs=2, name=f"wk{j}")
            wqj = mp.tile([P, ET, P], BF16, tag="wqj", bufs=2, name=f"wq{j}")
            nc.sync.dma_start(
                wkj[:], wk[:, j * P:(j + 1) * P].rearrange(
                    "(ko p) m -> p ko m", p=P))
            nc.sync.dma_start(
                wqj[:], wq[:, j * P:(j + 1) * P].rearrange(
                    "(ko p) m -> p ko m", p=P))
            wts[j] = (wkj, wqj)

        def emit_proj_chunk(j, which, dst, t0):
            wb = wts[j][which]
            uname = "kq"[which]
            ps = pspool.tile([P, 1024], FP32, tag="ps", bufs=3,
                             name=f"ps{uname}{j}_{t0}")
            # k-outer so the two 512-chunks of each k are adjacent with the
            # same stationary operand -> walrus merges them into one N=1024
            # matmul (~21% less chain time under the per-MM drain serial-
            # ization this compiler's disabled LDW-opt imposes)
            for k in range(ET):
                for n0 in (0, 512):
                    nc.tensor.matmul(
                        ps[:, n0:n0 + 512],
                        lhsT=wb[:, k, :],
                        rhs=xt_sb[:, k, t0 + n0:t0 + n0 + 512],
                        start=(k == 0), stop=(k == ET - 1))
            nc.vector.tensor_copy(out=dst[:, t0:t0 + 1024], in_=ps[:])

        def emit_proj_one(j, which, dst):
            for t0 in (0, 1024):
                emit_proj_chunk(j, which, dst, t0)

        # --- X^T (bf16 direct from host), chunked per k-tile for early start
        xt_sb = mp.tile([P, ET, T], BF16, tag="xtb", name="xtb")
        load_wkq(0)
        for k in range(ET):
            nc.sync.dma_start(xt_sb[:, k, :], xt[k * P:(k + 1) * P, :])
        wvb = mp.tile([P, ET, P * NPAIR], BF16, tag="wvb", name="wvb")
        nc.sync.dma_start(wvb[:], wv.rearrange("(ko p) m -> p ko m", p=P))
        nc.sync.dma_start(bub[:], bu.to_broadcast([P, E]))

        # pair-0 K/Q projection emitted first, chunk-interleaved so the
        # first scores only wait for (kt c0, qt c0)
        kq = {0: (mp.tile([P, T], BF16, tag="ktj", bufs=2, name="kt0"),
                  mp.tile([P, T], BF16, tag="qtj", bufs=2, name="qt0"))}
        emit_proj_chunk(0, 0, kq[0][0], 0)
        emit_proj_chunk(0, 1, kq[0][1], 0)
        emit_proj_chunk(0, 0, kq[0][0], 1024)
        emit_proj_chunk(0, 1, kq[0][1], 1024)

        # --- V projection -> vv[t, h, 0:64] token-major + ones column,
        # two token-tiles per PSUM tile
        for mt in range(0, KT, 2):
            ps = pspool.tile([P, 1024], FP32, tag="ps", bufs=3,
                             name=f"psv{mt}")
            for sub in range(2):
                for k in range(ET):
                    nc.tensor.matmul(
                        ps[:, sub * 512:(sub + 1) * 512],
                        lhsT=xt_sb[:, k, (mt + sub) * P:(mt + sub + 1) * P],
                        rhs=wvb[:, k, :],
                        start=(k == 0), stop=(k == ET - 1))
            nc.vector.tensor_copy(
                out=vv[:, mt:mt + 2, :, 0:S],
                in_=ps[:].rearrange("p (m h s) -> p m h s", m=2, s=S))
            nc.vector.memset(vv[:, mt:mt + 2, :, S:S + 1], 1.0)

        # output projection weights, DMA'd early so the tail never waits
        wub = mp.tile([P, FC, E], BF16, tag="wub", name="wub")
        nc.sync.dma_start(wub[:], wu.rearrange("(ko p) m -> p ko m", p=P))

        def emit_out_tile(m):
            # output projection token tile; contraction split into 64-row
            # halves on alternating row groups (halves stream concurrently,
            # ~2x a serial accumulation chain), packed into one ps tile
            ot = mp.tile([P, E], FP32, tag="ot", bufs=2, name=f"ot{m}")
            for n0 in (0, 512):
                ph = pspool.tile([P, 1024], FP32, tag="ps", bufs=3,
                                 name=f"po{m}_{n0}")
                for k in range(FC):
                    for h in range(2):
                        nc.tensor.matmul(
                            ph[:, h * 512:(h + 1) * 512],
                            lhsT=yt[h * S:(h + 1) * S, k,
                                    m * P:(m + 1) * P],
                            rhs=wub[h * S:(h + 1) * S, k, n0:n0 + 512],
                            start=(k == 0), stop=(k == FC - 1))
                tsum = mp.tile([P, 512], FP32, tag="osum", bufs=2,
                               name=f"os{m}_{n0}")
                nc.vector.tensor_add(out=tsum[:], in0=ph[:, 0:512],
                                     in1=bub[:, n0:n0 + 512])
                nc.vector.tensor_add(out=ot[:, n0:n0 + 512],
                                     in0=ph[:, 512:1024], in1=tsum[:])
            nc.sync.dma_start(out[m * P:(m + 1) * P, :], ot[:])

        # --- head pairs; per pair, 4 query-quarters of 512
        for j in range(NPAIR):
            ktj, qtj = kq.pop(j)
            for qq in range(NQ):
                q0 = qq * QW
                # AV accumulators: one PSUM bank per parity, held over the
                # full key loop (ones column gives the denominator as row 64)
                avs = [pspool.tile([P, QW], FP32, tag="av", bufs=2,
                                   name=f"av{j}_{qq}_{par}")
                       for par in range(2)]
                for i in range(KT):
                    ps = pspool.tile([P, 1024], FP32, tag="ps", bufs=3,
                                     name=f"s{j}_{qq}_{i}")
                    for par in range(2):
                        lo = par * S
                        nc.tensor.matmul(
                            ps[:, par * QW:(par + 1) * QW],
                            lhsT=ktj[lo:lo + S, i * P:(i + 1) * P],
                            rhs=qtj[lo:lo + S, q0:q0 + QW],
                            start=True, stop=True)
                    pt = mp.tile([P, 1024], BF16, tag="pt", bufs=16,
                                 name=f"p{j}_{qq}_{i}")
                    nc.scalar.activation(pt[:], ps[:], AF.Exp)
                    for par in range(2):
                        nc.tensor.matmul(
                            avs[par][0:S + 1, :],
                            lhsT=vv[:, i, 2 * j + par, :],
                            rhs=pt[:, par * QW:(par + 1) * QW],
                            start=(i == 0), stop=(i == KT - 1))
                # prefetch next pair's K/Q projection into the PE's slack,
                # one 1024-token chunk per quarter boundary (kt c0/c1 after
                # qq0/qq1, qt c0/c1 after qq2/qq3) so each block nearly
                # fits under the pending-exp backlog. qt c1 (q 1024..2047)
                # is only needed by (j+1, qq2), so the qq3 boundary is safe.
                if j + 1 < NPAIR:
                    if qq == 0:
                        load_wkq(j + 1)
                        kq[j + 1] = (
                            mp.tile([P, T], BF16, tag="ktj", bufs=2,
                                    name=f"kt{j+1}"),
                            mp.tile([P, T], BF16, tag="qtj", bufs=2,
                                    name=f"qt{j+1}"))
                        emit_proj_chunk(j + 1, 0, kq[j + 1][0], 0)
                    elif qq == 1:
                        emit_proj_chunk(j + 1, 0, kq[j + 1][0], 1024)
                    elif qq == 2:
                        emit_proj_chunk(j + 1, 1, kq[j + 1][1], 0)
                    elif qq == 3:
                        emit_proj_chunk(j + 1, 1, kq[j + 1][1], 1024)

                # normalize: evict AV banks, batch both parities' denominator
                # rows into a [128, 8] partition-packed reciprocal via a DRAM
                # bounce, broadcast back, multiply into yt
                yraws = []
                for par in range(2):
                    yraw = mp.tile([S + 1, QW], FP32, tag=f"yraw{par}",
                                   bufs=2, name=f"yraw{j}_{qq}_{par}")
                    nc.vector.tensor_copy(out=yraw[:], in_=avs[par][0:S + 1, :])
                    yraws.append(yraw)
                db = drampool.tile([1, 1024], FP32, tag="db", bufs=4,
                                   name=f"db{j}_{qq}")
                for par in range(2):
                    nc.sync.dma_start(db[:, par * QW:(par + 1) * QW],
                                      yraws[par][S:S + 1, :])
                rin = mp.tile([P, 8], FP32, tag="rin", bufs=2,
                              name=f"rin{j}_{qq}")
                nc.sync.dma_start(
                    rin[:], db[0:1, :].rearrange("a (p f) -> (a p) f", p=P))
                rcp = mp.tile([P, 8], FP32, tag="rcp", bufs=2,
                              name=f"rcp{j}_{qq}")
                nc.vector.reciprocal_approx_fast(out=rcp[:], in_=rin[:])
                db2 = drampool.tile([1, 1024], FP32, tag="db2", bufs=4,
                                    name=f"db2{j}_{qq}")
                nc.sync.dma_start(
                    db2[0:1, :].rearrange("a (p f) -> (a p) f", p=P), rcp[:])
                for par in range(2):
                    rbc = mp.tile([S, QW], FP32, tag="rbc", bufs=2,
                                  name=f"rbc{j}_{qq}_{par}")
                    nc.sync.dma_start(
                        rbc[:],
                        db2[0:1, par * QW:(par + 1) * QW].to_broadcast(
                            [S, QW]))
                    if par == 0:
                        nc.vector.tensor_mul(out=yt[0:S, j, q0:q0 + QW],
                                             in0=yraws[par][0:S, :],
                                             in1=rbc[:])
                    else:
                        tmp = mp.tile([S, QW], BF16, tag="tmp", bufs=2,
                                      name=f"tmp{j}_{qq}")
                        nc.vector.tensor_mul(out=tmp[:],
                                             in0=yraws[par][0:S, :],
                                             in1=rbc[:])
                        nc.sync.dma_start(yt[S:P, j, q0:q0 + QW], tmp[:])

        # --- output projection out[q, e'] = Y^T.T @ W_u^T + b_u (partial)
        for m in range(T // P):
            emit_out_tile(m)


_NC_CACHE = {}


def _get_nc():
    if "nc" not in _NC_CACHE:
        _NC_CACHE["nc"] = build_nc()
    return _NC_CACHE["nc"]


def make_in_maps(X, W_k, W_q, W_v, W_u, b_u):
    X = np.asarray(X, np.float32)
    b = X.shape[0]
    HW = P * NPAIR  # 512 features per head-half
    # pre-transpose, pre-scale, cast to bf16 on host (same numerics as the
    # on-device scale+cast it replaces)
    wk_t = (np.asarray(W_k, np.float32).T * SCALE).astype(BF16NP)
    wq_t = (np.asarray(W_q, np.float32).T * SCALE).astype(BF16NP)
    wv_t = (np.asarray(W_v, np.float32).T * SCALE).astype(BF16NP)
    wu_t = np.asarray(W_u, np.float32).T.astype(BF16NP)
    bu2 = np.ascontiguousarray(np.asarray(b_u, np.float32).reshape(1, E))
    bu_zero = np.zeros((1, E), np.float32)
    wk_s = [np.ascontiguousarray(wk_t[:, hh * HW:(hh + 1) * HW])
            for hh in range(2)]
    wq_s = [np.ascontiguousarray(wq_t[:, hh * HW:(hh + 1) * HW])
            for hh in range(2)]
    wv_s = [np.ascontiguousarray(wv_t[:, hh * HW:(hh + 1) * HW])
            for hh in range(2)]
    wu_s = [np.ascontiguousarray(wu_t[hh * HW:(hh + 1) * HW, :])
            for hh in range(2)]
    xts = [np.ascontiguousarray(X[bi].T).astype(BF16NP) for bi in range(b)]
    in_maps = []
    for c in range(N_CORES):
        bi, hh = c // 2, c % 2
        in_maps.append({
            "xt": xts[bi],
            "wk": wk_s[hh], "wq": wq_s[hh], "wv": wv_s[hh],
            "wu": wu_s[hh],
            "bu": bu2 if hh == 0 else bu_zero,
        })
    return in_maps


def run(inputs, trace=False, **kwargs):
    """Run on hardware; returns (full output, BassKernelResults)."""
    X = np.asarray(inputs["X"], np.float32)
    b, t, e = X.shape
    nc = _get_nc()
    in_maps = make_in_maps(X, inputs["W_k"], inputs["W_q"], inputs["W_v"],
                           inputs["W_u"], inputs["b_u"])
    res = run_bass_kernel_spmd(nc, in_maps, core_ids=list(range(N_CORES)),
                               trace=trace, **kwargs)
    full = np.empty((b, t, e), np.float32)
    for bi in range(b):
        full[bi] = res.results[2 * bi]["out"] + res.results[2 * bi + 1]["out"]
    return full, res


def kernel(**inputs):
    full, _ = run(inputs)
    return full


# revision 46
# speedup vs baseline: 1.2992x; 1.1811x over previous
# Multi-head attention kernel for Trainium2 (8 NeuronCores, SPMD).
#
# Problem (hardcoded): X[4, 2048, 1024], W_k/W_q/W_v/W_u[1024, 1024], b_u[1024]
#   K = (X @ W_k.T) * s ; Q = (X @ W_q.T) * s ; V = (X @ W_v.T) * s   (s = 1024**-0.25)
#   S = Q @ K.T per head (16 heads, head_dim 64); P = softmax(S); Y = P @ V
#   out = Y @ W_u.T + b_u
#
# Sharding: core c = (batch c//2, head-half c%2). Each core computes K/Q/V for
# its 8 heads over the full sequence of its batch and the matching 512-wide
# slice of the output projection, producing a PARTIAL output [2048, 1024]
# (feature-contraction split). Host unshard = add the two partials per batch
# (column-sharded projection reduce); b_u is added on the hh==0 core only.
#
# Design (from trace analysis): exp on the Scalar engine is the hard floor
# (~295us/core: 33.5M score elements at 1 elem/cycle/lane @1.2GHz, invariant
# under sharding), so the kernel keeps ACT saturated and fits the PE work
# underneath it. Weights arrive pre-scaled/pre-transposed/bf16 from the host.
# The attention q axis runs in 512-wide quarters with both heads of a pair
# packed into one [128, 1024] score PSUM tile so each EXP activation covers
# both heads (N=1024 amortizes the ~350-cycle ACTIVATE overhead). Softmax
# denominators ride as row 64 of the AV matmul (ones column in V); their
# reciprocal is computed partition-packed ([128, 8] via a DRAM bounce)
# instead of on a 1-partition row, which the DVE executes pathologically
# slowly. The output projection splits each contraction into two 64-row
# halves on alternating row groups (the hardware pulls row-disjoint weight
# loads ahead of in-flight matmuls, so the halves stream concurrently).
#
# Per-core layout (PE contracts on partitions):
#   X^T  [e, t]      bf16 from host
#   K^T/Q^T [128, t] per head pair: head A on partitions 0..63, head B 64..127
#   V    [t, h, 65]  token-major, 65th column = ones (softmax denominator)
#   Y^T  [128, 4, t] feature-major (pair -> partition block), normalized
#   out  [t, e]      partial projection, fp32

import numpy as np
import ml_dtypes

import concourse.bacc as bacc
import concourse.mybir as mybir
import concourse.tile as tile
from concourse.bass_utils import run_bass_kernel_spmd

FP32 = mybir.dt.float32
BF16 = mybir.dt.bfloat16
AF = mybir.ActivationFunctionType
BF16NP = ml_dtypes.bfloat16

P = 128
E = 1024            # embedding dim
T = 2048            # sequence length
HC = 8              # heads per core
S = 64              # head dim
ET = E // P         # 8 contraction tiles over e
KT = T // P         # 16 key tiles
NPAIR = HC // 2     # 4 head pairs per core
QW = 512            # query-quarter width
NQ = T // QW        # 4 query quarters
FC = NPAIR          # feature k-tiles for the output projection (4)
SCALE = float(E ** -0.25)

N_CORES = 8


def _chunks(total, step):
    return [(o, min(step, total - o)) for o in range(0, total, step)]


def build_nc():
    nc = bacc.Bacc("TRN2", target_bir_lowering=False, debug=False,
                   enable_asserts=False)

    xt = nc.dram_tensor("xt", [E, T], BF16, kind="ExternalInput").ap()
    wk = nc.dram_tensor("wk", [E, P * NPAIR], BF16, kind="ExternalInput").ap()
    wq = nc.dram_tensor("wq", [E, P * NPAIR], BF16, kind="ExternalInput").ap()
    wv = nc.dram_tensor("wv", [E, P * NPAIR], BF16, kind="ExternalInput").ap()
    wu = nc.dram_tensor("wu", [P * NPAIR, E], BF16, kind="ExternalInput").ap()
    bu = nc.dram_tensor("bu", [1, E], FP32, kind="ExternalInput").ap()
    out = nc.dram_tensor("out", [T, E], FP32, kind="ExternalOutput").ap()

    with tile.TileContext(nc) as tc:
        _build_kernel(tc, nc, xt, wk, wq, wv, wu, bu, out)
    nc.compile()
    return nc


def _build_kernel(tc, nc, xt, wk, wq, wv, wu, bu, out):
    with (
        tc.tile_pool(name="main", bufs=1) as mp,
        tc.tile_pool(name="psum", bufs=1, space="PSUM") as pspool,
        tc.tile_pool(name="dram", bufs=1, space="DRAM") as drampool,
    ):
        vv = mp.tile([P, KT, HC, S + 1], BF16, tag="vv", name="vv")
        yt = mp.tile([P, FC, T], BF16, tag="yt", name="yt")
        bub = mp.tile([P, E], FP32, tag="bub", name="bub")

        # --- per-pair K/Q weight load + projection (kt/qt [128, T]:
        # head 2j on partitions 0..63, head 2j+1 on 64..127)
        wts = {}

        def load_wkq(j):
            wkj = mp.tile([P, ET, P], BF16, tag="wkj", bufs=2, name=f"wk{j}")
            wqj = mp.tile([P, ET, P], BF16, tag="wqj", bufs=2, name=f"wq{j}")
            nc.sync.dma_start(
                wkj[:], wk[:, j * P:(j + 1) * P].rearrange(
                    "(ko p) m -> p ko m", p=P))
            nc.sync.dma_start(
                wqj[:], wq[:, j * P:(j + 1) * P].rearrange(
                    "(ko p) m -> p ko m", p=P))
            wts[j] = (wkj, wqj)

        def emit_proj_chunk(j, which, dst, t0):
            wb = wts[j][which]
            uname = "kq"[which]
            ps = pspool.tile([P, 1024], FP32, tag="ps", bufs=3,
                             name=f"ps{uname}{j}_{t0}")
            for n0 in (0, 512):
                for k in range(ET):
                    nc.tensor.matmul(
                        ps[:, n0:n0 + 512],
                        lhsT=wb[:, k, :],
                        rhs=xt_sb[:, k, t0 + n0:t0 + n0 + 512],
                        start=(k == 0), stop=(k == ET - 1))
            nc.vector.tensor_copy(out=dst[:, t0:t0 + 1024], in_=ps[:])

        def emit_proj_one(j, which, dst):
            for t0 in (0, 1024):
                emit_proj_chunk(j, which, dst, t0)

        # --- X^T (bf16 direct from host), chunked per k-tile for early start
        xt_sb = mp.tile([P, ET, T], BF16, tag="xtb", name="xtb")
        load_wkq(0)
        for k in range(ET):
            nc.sync.dma_start(xt_sb[:, k, :], xt[k * P:(k + 1) * P, :])
        wvb = mp.tile([P, ET, P * NPAIR], BF16, tag="wvb", name="wvb")
        nc.sync.dma_start(wvb[:], wv.rearrange("(ko p) m -> p ko m", p=P))
        nc.sync.dma_start(bub[:], bu.to_broadcast([P, E]))

        # pair-0 K/Q projection emitted first, chunk-interleaved so the
        # first scores only wait for (kt c0, qt c0)
        kq = {0: (mp.tile([P, T], BF16, tag="ktj", bufs=2, name="kt0"),
                  mp.tile([P, T], BF16, tag="qtj", bufs=2, name="qt0"))}
        emit_proj_chunk(0, 0, kq[0][0], 0)
        emit_proj_chunk(0, 1, kq[0][1], 0)
        emit_proj_chunk(0, 0, kq[0][0], 1024)
        emit_proj_chunk(0, 1, kq[0][1], 1024)

        # --- V projection -> vv[t, h, 0:64] token-major + ones column,
        # two token-tiles per PSUM tile
        for mt in range(0, KT, 2):
            ps = pspool.tile([P, 1024], FP32, tag="ps", bufs=3,
                             name=f"psv{mt}")
            for sub in range(2):
                for k in range(ET):
                    nc.tensor.matmul(
                        ps[:, sub * 512:(sub + 1) * 512],
                        lhsT=xt_sb[:, k, (mt + sub) * P:(mt + sub + 1) * P],
                        rhs=wvb[:, k, :],
                        start=(k == 0), stop=(k == ET - 1))
            nc.vector.tensor_copy(
                out=vv[:, mt:mt + 2, :, 0:S],
                in_=ps[:].rearrange("p (m h s) -> p m h s", m=2, s=S))
            nc.vector.memset(vv[:, mt:mt + 2, :, S:S + 1], 1.0)

        # output projection weights, DMA'd early so the tail never waits
        wub = mp.tile([P, FC, E], BF16, tag="wub", name="wub")
        nc.sync.dma_start(wub[:], wu.rearrange("(ko p) m -> p ko m", p=P))

        def emit_out_tile(m):
            # output projection token tile; contraction split into 64-row
            # halves on alternating row groups (halves stream concurrently,
            # ~2x a serial accumulation chain), packed into one ps tile
            ot = mp.tile([P, E], FP32, tag="ot", bufs=2, name=f"ot{m}")
            for n0 in (0, 512):
                ph = pspool.tile([P, 1024], FP32, tag="ps", bufs=3,
                                 name=f"po{m}_{n0}")
                for k in range(FC):
                    for h in range(2):
                        nc.tensor.matmul(
                            ph[:, h * 512:(h + 1) * 512],
                            lhsT=yt[h * S:(h + 1) * S, k,
                                    m * P:(m + 1) * P],
                            rhs=wub[h * S:(h + 1) * S, k, n0:n0 + 512],
                            start=(k == 0), stop=(k == FC - 1))
                tsum = mp.tile([P, 512], FP32, tag="osum", bufs=2,
                               name=f"os{m}_{n0}")
                nc.vector.tensor_add(out=tsum[:], in0=ph[:, 0:512],
                                     in1=bub[:, n0:n0 + 512])
                nc.vector.tensor_add(out=ot[:, n0:n0 + 512],
                                     in0=ph[:, 512:1024], in1=tsum[:])
            nc.sync.dma_start(out[m * P:(m + 1) * P, :], ot[:])

        # --- head pairs; per pair, 4 query-quarters of 512
        for j in range(NPAIR):
            ktj, qtj = kq.pop(j)
            for qq in range(NQ):
                q0 = qq * QW
                # AV accumulators: one PSUM bank per parity, held over the
                # full key loop (ones column gives the denominator as row 64)
                avs = [pspool.tile([P, QW], FP32, tag="av", bufs=2,
                                   name=f"av{j}_{qq}_{par}")
                       for par in range(2)]
                for i in range(KT):
                    ps = pspool.tile([P, 1024], FP32, tag="ps", bufs=3,
                                     name=f"s{j}_{qq}_{i}")
                    for par in range(2):
                        lo = par * S
                        nc.tensor.matmul(
                            ps[:, par * QW:(par + 1) * QW],
                            lhsT=ktj[lo:lo + S, i * P:(i + 1) * P],
                            rhs=qtj[lo:lo + S, q0:q0 + QW],
                            start=True, stop=True)
                    pt = mp.tile([P, 1024], BF16, tag="pt", bufs=16,
                                 name=f"p{j}_{qq}_{i}")
                    nc.scalar.activation(pt[:], ps[:], AF.Exp)
                    for par in range(2):
                        nc.tensor.matmul(
                            avs[par][0:S + 1, :],
                            lhsT=vv[:, i, 2 * j + par, :],
                            rhs=pt[:, par * QW:(par + 1) * QW],
                            start=(i == 0), stop=(i == KT - 1))
                # prefetch next pair's K/Q projection into the PE's slack,
                # split across two quarter boundaries (kt after qq0, qt
                # after qq1) so each block is half the size
                if j + 1 < NPAIR:
                    if qq == 0:
                        load_wkq(j + 1)
                        kq[j + 1] = (
                            mp.tile([P, T], BF16, tag="ktj", bufs=2,
                                    name=f"kt{j+1}"),
                            mp.tile([P, T], BF16, tag="qtj", bufs=2,
                                    name=f"qt{j+1}"))
                        emit_proj_one(j + 1, 0, kq[j + 1][0])
                    elif qq == 1:
                        emit_proj_one(j + 1, 1, kq[j + 1][1])

                # normalize: evict AV banks, batch both parities' denominator
                # rows into a [128, 8] partition-packed reciprocal via a DRAM
                # bounce, broadcast back, multiply into yt
                yraws = []
                for par in range(2):
                    yraw = mp.tile([S + 1, QW], FP32, tag=f"yraw{par}",
                                   bufs=2, name=f"yraw{j}_{qq}_{par}")
                    nc.vector.tensor_copy(out=yraw[:], in_=avs[par][0:S + 1, :])
                    yraws.append(yraw)
                db = drampool.tile([1, 1024], FP32, tag="db", bufs=4,
                                   name=f"db{j}_{qq}")
                for par in range(2):
                    nc.sync.dma_start(db[:, par * QW:(par + 1) * QW],
                                      yraws[par][S:S + 1, :])
                rin = mp.tile([P, 8], FP32, tag="rin", bufs=2,
                              name=f"rin{j}_{qq}")
                nc.sync.dma_start(
                    rin[:], db[0:1, :].rearrange("a (p f) -> (a p) f", p=P))
                rcp = mp.tile([P, 8], FP32, tag="rcp", bufs=2,
                              name=f"rcp{j}_{qq}")
                nc.vector.reciprocal_approx_fast(out=rcp[:], in_=rin[:])
                db2 = drampool.tile([1, 1024], FP32, tag="db2", bufs=4,
                                    name=f"db2{j}_{qq}")
                nc.sync.dma_start(
                    db2[0:1, :].rearrange("a (p f) -> (a p) f", p=P), rcp[:])
                for par in range(2):
                    rbc = mp.tile([S, QW], FP32, tag="rbc", bufs=2,
                                  name=f"rbc{j}_{qq}_{par}")
                    nc.sync.dma_start(
                        rbc[:],
                        db2[0:1, par * QW:(par + 1) * QW].to_broadcast(
                            [S, QW]))
                    if par == 0:
                        nc.vector.tensor_mul(out=yt[0:S, j, q0:q0 + QW],
                                             in0=yraws[par][0:S, :],
                                             in1=rbc[:])
                    else:
                        tmp = mp.tile([S, QW], BF16, tag="tmp", bufs=2,
                                      name=f"tmp{j}_{qq}")
                        nc.vector.tensor_mul(out=tmp[:],
                                             in0=yraws[par][0:S, :],
                                             in1=rbc[:])
                        nc.sync.dma_start(yt[S:P, j, q0:q0 + QW], tmp[:])

        # --- output projection out[q, e'] = Y^T.T @ W_u^T + b_u (partial)
        for m in range(T // P):
            emit_out_tile(m)


_NC_CACHE = {}


def _get_nc():
    if "nc" not in _NC_CACHE:
        _NC_CACHE["nc"] = build_nc()
    return _NC_CACHE["nc"]


def make_in_maps(X, W_k, W_q, W_v, W_u, b_u):
    X = np.asarray(X, np.float32)
    b = X.shape[0]
    HW = P * NPAIR  # 512 features per head-half
    # pre-transpose, pre-scale, cast to bf16 on host (same numerics as the
    # on-device scale+cast it replaces)
    wk_t = (np.asarray(W_k, np.float32).T * SCALE).astype(BF16NP)
    wq_t = (np.asarray(W_q, np.float32).T * SCALE).astype(BF16NP)
    wv_t = (np.asarray(W_v, np.float32).T * SCALE).astype(BF16NP)
    wu_t = np.asarray(W_u, np.float32).T.astype(BF16NP)
    bu2 = np.ascontiguousarray(np.asarray(b_u, np.float32).reshape(1, E))
    bu_zero = np.zeros((1, E), np.float32)
    wk_s = [np.ascontiguousarray(wk_t[:, hh * HW:(hh + 1) * HW])
            for hh in range(2)]
    wq_s = [np.ascontiguousarray(wq_t[:, hh * HW:(hh + 1) * HW])
            for hh in range(2)]
    wv_s = [np.ascontiguousarray(wv_t[:, hh * HW:(hh + 1) * HW])
            for hh in range(2)]
    wu_s = [np.ascontiguousarray(wu_t[hh * HW:(hh + 1) * HW, :])
            for hh in range(2)]
    xts = [np.ascontiguousarray(X[bi].T).astype(BF16NP) for bi in range(b)]
    in_maps = []
    for c in range(N_CORES):
        bi, hh = c // 2, c % 2
        in_maps.append({
            "xt": xts[bi],
            "wk": wk_s[hh], "wq": wq_s[hh], "wv": wv_s[hh],
            "wu": wu_s[hh],
            "bu": bu2 if hh == 0 else bu_zero,
        })
    return in_maps


def run(inputs, trace=False, **kwargs):
    """Run on hardware; returns (full output, BassKernelResults)."""
    X = np.asarray(inputs["X"], np.float32)
    b, t, e = X.shape
    nc = _get_nc()
    in_maps = make_in_maps(X, inputs["W_k"], inputs["W_q"], inputs["W_v"],
                           inputs["W_u"], inputs["b_u"])
    res = run_bass_kernel_spmd(nc, in_maps, core_ids=list(range(N_CORES)),
                               trace=trace, **kwargs)
    full = np.empty((b, t, e), np.float32)
    for bi in range(b):
        full[bi] = res.results[2 * bi]["out"] + res.results[2 * bi + 1]["out"]
    return full, res


def kernel(**inputs):
    full, _ = run(inputs)
    return full
